# revision 42
# baseline (speedup 1.0000x reference)
"""Multi-head attention (B=4, S=2048, D=1024, H=16) on 8 Trainium2 NeuronCores.

Sharding: core c -> batch c//2, head-group c%2 (8 heads = 512 dims each).
Each core computes qkv projection, softmax attention and its partial
out-projection (Megatron row-split of w_out); the host sums core pairs.

All projection/attention operands are bf16 with fp32 PSUM accumulation
(x, qkv weights, qT/kT, v, exp(scores)); only the out-projection stays
float32r.  bf16 halves DMA bytes (the DMA engines are packet-rate bound:
~288ns per <=2KB line, so all big transfers use >=2KB lines) and makes
every hot-loop LDWEIGHTS a standalone fast-load that hides behind
matmuls.  x lives resident in SBUF (4MB bf16) so phase-B fillers do no
DMA.  Softmax needs no max-subtraction (scores ~ N(0,1)); denominators
come free from an augmented ones-column in V; the reciprocal runs on a
[128,4] DRAM-reshaped layout (DVE reciprocal cost scales with free
size), and the normalization multiply runs on the idle GPSIMD.

Schedule (profiled 382us, exp-ACT-bound):
  - pass 1 (~60us, DMA-paced): one sweep over resident x computes v (all
    heads) + q/k (pair 0), six back-to-back matmuls per x tile; psum
    drains double-buffered so chunk boundaries never stall the PE.
  - phase B (~274us, ACT ~99% busy): per head-pair, scores (row-tiled
    2-head pack) -> one exp per [128,1024] psum tile -> pv accumulation
    lagging 2 iterations (pv LDWEIGHTS carries no pending wait); filler
    matmuls project the next pair's q/k from resident x in the PE slack,
    finishing 2 iterations before the qu boundary.
  - tail (~29us): transposed out-projection (wo stationary, output [D,S]
    bf16, host transposes back), PSUM from freed attention tags (no pool
    barrier), bias adds alternating ACT/DVE, paired [128,1024] output
    DMAs.
"""

import numpy as np

B, S, D, H = 4, 2048, 1024, 16
HD = D // H          # 64
HG = H // 2          # 8 heads per core
DG = HG * HD         # 512 local head-cat dims
SCALE = HD ** -0.5   # folded into wq host-side
NCORES = 8

_CACHE = {}


# --------------------------------------------------------------------------
# wait splitting: this toolchain's walrus rejects >1 sync wait per instruction
# on some paths; move excess semaphore waits onto same-engine NoOps.
# --------------------------------------------------------------------------
def _split_excess_waits(nc, max_waits=1):
    import bass_rust
    import concourse.mybir as mybir

    ctr = [0]
    for fn in nc.m.functions:
        for bb in fn.blocks:
            insts = list(bb.instructions)
            out = []
            changed = False
            for inst in insts:
                si = inst.sync_info
                waits = list(si.on_wait) if si is not None and si.on_wait else []
                sem_waits = [w for w in waits if w.sync_type == "semaphore"]
                other = [w for w in waits if w.sync_type != "semaphore"]
                budget = max_waits - len(other)
                if len(sem_waits) > budget and budget >= 1:
                    head, keep = sem_waits[:-budget], sem_waits[-budget:]
                    chunks = [
                        head[i : i + max_waits]
                        for i in range(0, len(head), max_waits)
                    ]
                    for ch in chunks:
                        nop = mybir.InstNoOp(
                            name=f"wsplit-{ctr[0]}",
                            opcode="NoOp",
                            engine=inst.engine,
                            ins=[],
                            outs=[],
                        )
                        nop.sync_info = bass_rust.SyncInfo(on_wait=ch, on_update=[])
                        ctr[0] += 1
                        out.append(nop)
                    inst.sync_info = bass_rust.SyncInfo(
                        on_wait=other + keep,
                        on_update=list(si.on_update) if si.on_update else [],
                    )
                    changed = True
                out.append(inst)
            if changed:
                bb.instructions = out


# --------------------------------------------------------------------------
# device program (identical on all 8 cores)
# --------------------------------------------------------------------------
def _build(split_waits=True):
    import concourse.bass as bass
    import concourse.tile as tile
    import concourse.mybir as mybir

    F32 = mybir.dt.float32
    F32R = mybir.dt.float32r
    BF16 = mybir.dt.bfloat16
    EXP = mybir.ActivationFunctionType.Exp
    IDENT = mybir.ActivationFunctionType.Identity
    ts = bass.ts

    nc = bass.Bass()

    xb = nc.dram_tensor("xb", [D, S], BF16, kind="ExternalInput")
    wqk = nc.dram_tensor("wqk", [D, 2 * DG], BF16, kind="ExternalInput")
    wv = nc.dram_tensor("wv", [D, DG], BF16, kind="ExternalInput")
    bqk = nc.dram_tensor("bqk", [128, 8], F32, kind="ExternalInput")
    bv = nc.dram_tensor("bv", [128, DG], F32, kind="ExternalInput")
    wo = nc.dram_tensor("wo", [DG, D], F32R, kind="ExternalInput")
    bo = nc.dram_tensor("bo", [128, D // 128], F32, kind="ExternalInput")
    outp = nc.dram_tensor("outp", [D, S], BF16, kind="ExternalOutput")

    NSQT = S // 128          # 16 sq/sk tiles of 128
    NDT = D // 128           # 8 contraction tiles
    NPAIR = HG // 2          # 4 head pairs
    VW = HD + 1              # 65: v columns + ones column per head
    SQQ = 512                # sq quarter per pv accumulation

    with tile.TileContext(nc) as tc:
        with (
            tc.tile_pool(name="bias", bufs=1) as bias_pool,
            tc.tile_pool(name="vaug", bufs=1) as v_pool,
            tc.tile_pool(name="oT", bufs=1) as oT_pool,
            tc.tile_pool(name="qkp", bufs=1) as qkp,      # rotating qT/kT slots
            tc.tile_pool(name="wqkp", bufs=1) as wqkp,
            tc.tile_pool(name="xs2", bufs=1) as xs2,      # resident bf16 x [D,S]
            tc.tile_pool(name="psqk", bufs=1, space="PSUM") as ps_qk,  # pa,pb
        ):
            bqk_t = bias_pool.tile([128, 8], F32)
            nc.gpsimd.dma_start(bqk_t[:], bqk[:, :])
            bv_t = bias_pool.tile([128, DG], F32)
            nc.gpsimd.dma_start(bv_t[:], bv[:, :])
            ones8_f = bias_pool.tile([128, 8], F32)
            nc.vector.memset(ones8_f[:], 1.0)
            ones8 = bias_pool.tile([128, 8], BF16)
            nc.vector.tensor_copy(ones8[:], ones8_f[:])
            # preload the exp table set during pass 1 (one-time ~2.7us)
            actwarm = bias_pool.tile([1, 1], F32)
            nc.scalar.activation(actwarm[:], ones8_f[0:1, 0:1], EXP)

            # wqk_t[d]: cols 0:512 = wq (SCALE folded), 512:1024 = wk
            wqk_t = [
                wqkp.tile([128, 2 * DG], BF16, name=f"wqk{d}", tag=f"wqk{d}")
                for d in range(NDT)
            ]

            v_t = [v_pool.tile([128, HG * VW], BF16, name=f"v{s}", tag=f"v{s}") for s in range(NSQT)]
            oT_t = [oT_pool.tile([128, S], F32R, name=f"oT{p}", tag=f"oT{p}") for p in range(NPAIR)]
            qT_t = [qkp.tile([128, S], BF16, name=f"qT{i}", tag=f"qT{i}") for i in range(2)]
            kT_t = [qkp.tile([128, S], BF16, name=f"kT{i}", tag=f"kT{i}") for i in range(2)]

            # ------- pass 1: single sweep over x -> v(all) + qk(pair0) -------
            # x arrives as [128,1024] bf16 tiles (2KB DMA lines), each
            # resident across the two 512-col chunks that consume it; per
            # chunk+d we issue 6 back-to-back matmuls.  PSUM: gq/gk double-
            # buffered across chunks (pa,pb | pg,ph) + 4 v banks = 8.
            with (
                tc.tile_pool(name="wvp", bufs=1) as wvp,
                tc.tile_pool(name="psv", bufs=1, space="PSUM") as ps_v,
                tc.tile_pool(name="psg", bufs=1, space="PSUM") as ps_g,
            ):
                wv_t = [wvp.tile([128, DG], BF16, name=f"wv{d}", tag=f"wv{d}") for d in range(NDT)]
                # x stays resident in SBUF for the whole kernel (the phase-B
                # fillers then need no DMA at all); weight and x loads are
                # emitted d-interleaved in demand order across both hw queues
                xr_t = [
                    [
                        xs2.tile([128, 1024], BF16, name=f"x2_{cc}_{d}", tag=f"x{cc}_{d}")
                        for d in range(NDT)
                    ]
                    for cc in range(S // 1024)
                ]
                for d in range(NDT):
                    nc.sync.dma_start(wqk_t[d][:], wqk[ts(d, 128), :])
                    nc.scalar.dma_start(wv_t[d][:], wv[ts(d, 128), :])
                    nc.scalar.dma_start(xr_t[0][d][:], xb[ts(d, 128), ts(0, 1024)])
                    nc.sync.dma_start(xr_t[1][d][:], xb[ts(d, 128), ts(1, 1024)])

                for cc in range(S // 1024):
                    x2_t = xr_t[cc]
                    for sub in range(2):
                        ci = 2 * cc + sub
                        if ci % 2 == 0:
                            gq = ps_qk.tile([128, 512], F32, name="gq", tag="pa")
                            gk = ps_qk.tile([128, 512], F32, name="gk", tag="pb")
                        else:
                            gq = ps_g.tile([128, 512], F32, name="gq", tag="pg")
                            gk = ps_g.tile([128, 512], F32, name="gk", tag="ph")
                        psv = [
                            ps_v.tile([128, DG], F32, name="psv", tag=t)
                            for t in ("pc", "pd", "pe", "pf")
                        ]
                        for d in range(NDT):
                            xch = x2_t[d][:, ts(sub, 512)]
                            nc.tensor.matmul(
                                gq[:], wqk_t[d][:, 0:128], xch,
                                start=(d == 0), stop=(d == NDT - 1),
                            )
                            nc.tensor.matmul(
                                gk[:], wqk_t[d][:, DG : DG + 128], xch,
                                start=(d == 0), stop=(d == NDT - 1),
                            )
                            for si in range(4):
                                nc.tensor.matmul(
                                    psv[si][:], xch[:, ts(si, 128)], wv_t[d][:],
                                    start=(d == 0), stop=(d == NDT - 1),
                                )
                        # drains: v-aug first (next chunk's psv matmuls wait
                        # on these); qk bias adds last (banks double-buffered)
                        for si in range(4):
                            s = 4 * ci + si
                            vap = v_t[s][:].rearrange("p (h e) -> p h e", e=VW)
                            nc.vector.tensor_add(
                                vap[:, :, 0:HD],
                                psv[si][:].rearrange("p (h e) -> p h e", e=HD),
                                bv_t[:].rearrange("p (h e) -> p h e", e=HD),
                            )
                            nc.gpsimd.tensor_copy(
                                vap[:, :, HD : HD + 1], ones8[:, :, None]
                            )
                        nc.vector.tensor_scalar_add(
                            qT_t[0][:, ts(ci, 512)], gq[:], bqk_t[:, 0:1]
                        )
                        nc.vector.tensor_scalar_add(
                            kT_t[0][:, ts(ci, 512)], gk[:], bqk_t[:, 4:5]
                        )

            # ---------------- phase B: attention + fillers + tail ---------
            # Two heads share one [128,1024] scores psum tile; one exp covers
            # both heads.  pv matmuls are software-pipelined one step behind.
            # The next pair's q/k projection fills the PE slack under the
            # ACT-bound exp stream; x tiles are re-read as [128,1024] bf16
            # spanning two qu's.  The out-projection tail lives inside this
            # pool scope (reusing attention PSUM tags) so no pool-exit
            # barrier precedes it.
            with (
                tc.tile_pool(name="pt", bufs=3) as ptp,
                tc.tile_pool(name="scp", bufs=2, space="PSUM") as scp,
                tc.tile_pool(name="pvp", bufs=1, space="PSUM") as pvp,
                tc.tile_pool(name="nrm", bufs=1) as nrm,
                tc.tile_pool(name="pvs", bufs=1) as pvsp,
                tc.tile_pool(name="rs", bufs=4, space="DRAM") as rsp,
                tc.tile_pool(name="w3", bufs=1) as w3,
                tc.tile_pool(name="outb", bufs=3) as outb,
            ):
                wo_t = [
                    w3.tile([128, D], F32R, name=f"wo{pp}", tag=f"wo{pp}")
                    for pp in range(NPAIR)
                ]
                for pr in range(NPAIR):
                    h0, h1 = 2 * pr, 2 * pr + 1
                    qTc, kTc = qT_t[pr % 2], kT_t[pr % 2]
                    nxt = pr + 1 if pr + 1 < NPAIR else None
                    if pr == 2:
                        # wo arrives during pr=2 via the idle GPSIMD SWDGE
                        # (keeping the sync queue free for filler x chunks)
                        for pp in range(NPAIR):
                            nc.gpsimd.dma_start(wo_t[pp][:], wo[ts(pp, 128), :])
                    for qu in range(S // SQQ):
                        qs = slice(qu * SQQ, (qu + 1) * SQQ)
                        pv0 = pvp.tile([VW, SQQ], F32, name="pv0", tag="pv0")
                        pv1 = pvp.tile([VW, SQQ], F32, name="pv1", tag="pv1")
                        if nxt is not None:
                            gq = ps_qk.tile([128, 512], F32, name="gq", tag="pa")
                            gk = ps_qk.tile([128, 512], F32, name="gk", tag="pb")
                        # pv matmuls run 2 iterations behind their exp so the
                        # pv LDWEIGHTS carries no pending wait and hides
                        # behind in-flight matmuls
                        pt_q = []
                        for s in range(NSQT):
                            sc = scp.tile([128, 2 * SQQ], F32, name="sc", tag="sc")
                            nc.tensor.matmul(
                                sc[:, 0:SQQ],
                                kTc[0:HD, ts(s, 128)],
                                qTc[0:HD, qs],
                                start=True, stop=True,
                            )
                            nc.tensor.matmul(
                                sc[:, SQQ : 2 * SQQ],
                                kTc[HD:128, ts(s, 128)],
                                qTc[HD:128, qs],
                                start=True, stop=True,
                            )
                            pt = ptp.tile([128, 2 * SQQ], BF16, name="pt", tag="pt")
                            nc.scalar.activation(pt[:], sc[:], EXP)
                            if len(pt_q) == 2:
                                pp_, ps_ = pt_q.pop(0)
                                nc.tensor.matmul(
                                    pv0[:], v_t[ps_][:, h0 * VW : (h0 + 1) * VW],
                                    pp_[:, 0:SQQ],
                                    start=(ps_ == 0), stop=False,
                                )
                                nc.tensor.matmul(
                                    pv1[:], v_t[ps_][:, h1 * VW : (h1 + 1) * VW],
                                    pp_[:, SQQ : 2 * SQQ],
                                    start=(ps_ == 0), stop=False,
                                )
                            if nxt is not None:
                                # filler: qk projection for the next pair,
                                # 1 matmul/iter from the resident x tiles,
                                # finishing (with bias adds) at s==14
                                if s < 12:
                                    d = s // 2
                                    xch = xr_t[qu // 2][d][:, ts(qu % 2, 512)]
                                    if s % 2 == 0:
                                        nc.tensor.matmul(
                                            gq[:], wqk_t[d][:, ts(nxt, 128)], xch,
                                            start=(d == 0), stop=False,
                                        )
                                    else:
                                        nc.tensor.matmul(
                                            gk[:],
                                            wqk_t[d][:, DG + nxt * 128 : DG + (nxt + 1) * 128],
                                            xch,
                                            start=(d == 0), stop=False,
                                        )
                                elif s in (12, 13):
                                    d = s - 6
                                    xch = xr_t[qu // 2][d][:, ts(qu % 2, 512)]
                                    nc.tensor.matmul(
                                        gq[:], wqk_t[d][:, ts(nxt, 128)], xch,
                                        start=False, stop=(d == NDT - 1),
                                    )
                                    nc.tensor.matmul(
                                        gk[:],
                                        wqk_t[d][:, DG + nxt * 128 : DG + (nxt + 1) * 128],
                                        xch,
                                        start=False, stop=(d == NDT - 1),
                                    )
                                    if s == 13:
                                        nc.vector.tensor_scalar_add(
                                            qT_t[nxt % 2][:, ts(qu, 512)], gq[:],
                                            bqk_t[:, nxt : nxt + 1],
                                        )
                                        nc.vector.tensor_scalar_add(
                                            kT_t[nxt % 2][:, ts(qu, 512)], gk[:],
                                            bqk_t[:, 4 + nxt : 5 + nxt],
                                        )
                            pt_q.append((pt, s))
                        for pp_, ps_ in pt_q:
                            nc.tensor.matmul(
                                pv0[:], v_t[ps_][:, h0 * VW : (h0 + 1) * VW],
                                pp_[:, 0:SQQ],
                                start=False, stop=(ps_ == NSQT - 1),
                            )
                            nc.tensor.matmul(
                                pv1[:], v_t[ps_][:, h1 * VW : (h1 + 1) * VW],
                                pp_[:, SQQ : 2 * SQQ],
                                start=False, stop=(ps_ == NSQT - 1),
                            )
                        # free pv banks via psum->sbuf copy, then normalize.
                        # The denominator row [1,512] is reshaped to [128,4]
                        # through DRAM so the DVE reciprocal costs ~0.1us
                        # instead of 3.3us; the multiply runs on GPSIMD.
                        for hh, pvx, row in ((0, pv0, 0), (1, pv1, HD)):
                            pvs = pvsp.tile([VW, SQQ], F32, name=f"pvs{hh}", tag=f"pvs{hh}")
                            nc.vector.tensor_copy(pvs[:], pvx[:])
                            rsd = rsp.tile([1, SQQ], F32, name=f"rsd{hh}", tag=f"rsd{hh}")
                            nc.sync.dma_start(rsd[:], pvs[HD : HD + 1, :])
                            dent = nrm.tile([128, 4], F32, name=f"dent{hh}", tag=f"dent{hh}")
                            nc.sync.dma_start(dent[:], rsd[:])
                            rcd = nrm.tile([128, 4], F32, name=f"rcd{hh}", tag=f"rcd{hh}")
                            nc.vector.reciprocal(rcd[:], dent[:])
                            rs2 = rsp.tile([1, SQQ], F32, name=f"rs2{hh}", tag=f"rs2{hh}")
                            nc.sync.dma_start(rs2[:], rcd[:])
                            bcs = nrm.tile([HD, SQQ], F32, name=f"bcs{hh}", tag=f"bcs{hh}")
                            nc.sync.dma_start(bcs[:], rs2[:].broadcast_to([HD, SQQ]))
                            nc.gpsimd.tensor_mul(
                                oT_t[pr][row : row + HD, qs], pvs[0:HD, :], bcs[:]
                            )

                # ---------------- tail: out projection (transposed) -------
                # outT[j,q] = sum_pp wo_pp[:,j]^T @ oT_pp[:,q]: wo stationary
                # (reused over 4 moving q-chunks), bias as per-partition
                # scalar on the now-idle ACT engine, host transposes [D,S].
                # PSUM comes from freed attention tags; q-chunk 3 (which
                # waits on the last qu's normalization) goes last.
                bo_t = outb.tile([128, NDT], F32, name="bo_t", tag="bo_t")
                nc.gpsimd.dma_start(bo_t[:], bo[:, :])
                po_slots = [
                    (scp, "sc"), (ps_qk, "pa"), (ps_qk, "pb"),
                    (pvp, "pv0"), (pvp, "pv1"), (scp, "sc"),
                ]
                nunit = 0

                def _unit2(j, tcpair):
                    # two q-chunks -> one [128,1024] bf16 tile -> one DMA
                    # (2KB lines; bf16 halves the 8MB output stream)
                    nonlocal nunit
                    ob = outb.tile([128, 1024], BF16, name=f"ob{j}_{tcpair}", tag="ob")
                    for half in range(2):
                        tc_ = 2 * tcpair + half
                        pool, tag = po_slots[nunit % len(po_slots)]
                        po = pool.tile([128, 512], F32, name=f"po{j}_{tc_}", tag=tag)
                        for pp in range(NPAIR):
                            nc.tensor.matmul(
                                po[:],
                                wo_t[pp][:, ts(j, 128)],
                                oT_t[pp][:, ts(tc_, 512)],
                                start=(pp == 0), stop=(pp == NPAIR - 1),
                            )
                        if nunit % 2 == 0:
                            nc.scalar.activation(
                                ob[:, ts(half, 512)], po[:], IDENT,
                                bias=bo_t[:, j : j + 1],
                            )
                        else:
                            nc.vector.tensor_scalar_add(
                                ob[:, ts(half, 512)], po[:], bo_t[:, j : j + 1]
                            )
                        nunit += 1
                    nc.sync.dma_start(outp[ts(j, 128), ts(tcpair, 1024)], ob[:])

                for j in range(NDT):
                    _unit2(j, 0)
                for j in range(NDT):
                    _unit2(j, 1)

    if split_waits:
        _split_excess_waits(nc, max_waits=1)
    return nc


def _get_nc():
    if "nc" not in _CACHE:
        _CACHE["nc"] = _build()
    return _CACHE["nc"]


# --------------------------------------------------------------------------
# host entry point
# --------------------------------------------------------------------------
def _shard_inputs(x, w_qkv, b_qkv, w_out, b_out):
    import ml_dtypes

    f = np.float32
    bf = np.dtype(ml_dtypes.bfloat16)
    x = np.asarray(x, f)
    w_qkv = np.asarray(w_qkv, f)
    b_qkv = np.asarray(b_qkv, f)
    w_out = np.asarray(w_out, f)
    b_out = np.asarray(b_out, f)
    in_maps = []
    for c in range(NCORES):
        b, g = divmod(c, 2)
        cols = slice(DG * g, DG * (g + 1))
        wq_c = w_qkv[:, 0 * D :][:, cols][:, :DG] * np.float32(SCALE)
        wk_c = w_qkv[:, D : 2 * D][:, cols]
        wqk_c = np.ascontiguousarray(
            np.concatenate([wq_c, wk_c], axis=1).astype(bf)
        )
        wv_c = np.ascontiguousarray(w_qkv[:, 2 * D :][:, cols].astype(bf))
        bq_c = (b_qkv[0 * D : 1 * D][cols] * np.float32(SCALE)).reshape(4, 128).T
        bk_c = b_qkv[D : 2 * D][cols].reshape(4, 128).T
        bqk_c = np.ascontiguousarray(np.concatenate([bq_c, bk_c], axis=1), f)
        bv_c = np.ascontiguousarray(np.tile(b_qkv[2 * D :][cols], (128, 1)), f)
        wo_c = np.ascontiguousarray(w_out[DG * g : DG * (g + 1), :])
        bo_c = (
            np.ascontiguousarray(b_out.reshape(D // 128, 128).T, f)
            if g == 0
            else np.zeros((128, D // 128), f)
        )
        in_maps.append(
            {
                "xb": np.ascontiguousarray(x[b].T.astype(bf)),
                "wqk": wqk_c,
                "wv": wv_c,
                "bqk": bqk_c,
                "bv": bv_c,
                "wo": wo_c,
                "bo": bo_c,
            }
        )
    return in_maps


def _patch_ldw_opt():
    """Flip walrus --enable-ldw-opt to true (dedupe repeated LDWEIGHTS for
    consecutive same-stationary matmuls). Off by default: the bf16 matmuls
    now lower to standalone InstLdweights, which walrus rejects under
    ldw-opt. Controlled by KERNEL_LDW_OPT env."""
    import os
    if os.environ.get("KERNEL_LDW_OPT", "0") != "1":
        return
    if _CACHE.get("ldw_patched"):
        return
    import concourse.bass_utils as bu

    orig = bu.run_command

    def run_command_ldw(argv, **kwargs):
        argv = [a.replace("--enable-ldw-opt=false", "--enable-ldw-opt=true")
                if isinstance(a, str) else a for a in argv]
        return orig(argv, **kwargs)

    bu.run_command = run_command_ldw
    _CACHE["ldw_patched"] = True


def kernel(x, w_qkv, b_qkv, w_out, b_out, _trace=False, _trace_kwargs=None):
    from concourse.bass_utils import run_bass_kernel_spmd

    _patch_ldw_opt()
    nc = _get_nc()
    in_maps = _shard_inputs(x, w_qkv, b_qkv, w_out, b_out)
    kw = {}
    if _trace:
        kw["trace"] = True
        kw.update(_trace_kwargs or {})
    res = run_bass_kernel_spmd(nc, in_maps, core_ids=list(range(NCORES)), **kw)
    _CACHE["last_result"] = res
    # [D, S] bf16 per core
    parts = [np.asarray(r["outp"], dtype=np.float32) for r in res.results]
    out = np.stack([(parts[2 * b] + parts[2 * b + 1]).T for b in range(B)])
    return np.ascontiguousarray(out, np.float32)


# revision 43
# speedup vs baseline: 1.0237x; 1.0237x over previous
"""Multi-head attention (B=4, S=2048, D=1024, H=16) on 8 Trainium2 NeuronCores.

Sharding: core c -> batch c//2, head-group c%2 (8 heads = 512 dims each).
Each core computes qkv projection, softmax attention and its partial
out-projection (Megatron row-split of w_out); the host sums core pairs.

All projection/attention operands are bf16 with fp32 PSUM accumulation
(x, qkv weights, qT/kT, v, exp(scores)); only the out-projection stays
float32r.  bf16 halves DMA bytes (the DMA engines are packet-rate bound:
~288ns per <=2KB line, so all big transfers use >=2KB lines) and makes
every hot-loop LDWEIGHTS a standalone fast-load that hides behind
matmuls.  x lives resident in SBUF (4MB bf16) so phase-B fillers do no
DMA.  Softmax needs no max-subtraction (scores ~ N(0,1)); denominators
come free from an augmented ones-column in V; the reciprocal runs on a
[128,4] DRAM-reshaped layout (DVE reciprocal cost scales with free
size), and the normalization multiply runs on the idle GPSIMD.

Schedule (profiled 382us, exp-ACT-bound):
  - pass 1 (~60us, DMA-paced): one sweep over resident x computes v (all
    heads) + q/k (pair 0), six back-to-back matmuls per x tile; psum
    drains double-buffered so chunk boundaries never stall the PE.
  - phase B (~274us, ACT ~99% busy): per head-pair, scores (row-tiled
    2-head pack) -> one exp per [128,1024] psum tile -> pv accumulation
    lagging 2 iterations (pv LDWEIGHTS carries no pending wait); filler
    matmuls project the next pair's q/k from resident x in the PE slack,
    finishing 2 iterations before the qu boundary.
  - tail (~29us): transposed out-projection (wo stationary, output [D,S]
    bf16, host transposes back), PSUM from freed attention tags (no pool
    barrier), bias adds alternating ACT/DVE, paired [128,1024] output
    DMAs.
"""

import numpy as np

B, S, D, H = 4, 2048, 1024, 16
HD = D // H          # 64
HG = H // 2          # 8 heads per core
DG = HG * HD         # 512 local head-cat dims
SCALE = HD ** -0.5   # folded into wq host-side
NCORES = 8

_CACHE = {}


# --------------------------------------------------------------------------
# wait splitting: this toolchain's walrus rejects >1 sync wait per instruction
# on some paths; move excess semaphore waits onto same-engine NoOps.
# --------------------------------------------------------------------------
def _split_excess_waits(nc, max_waits=1):
    import bass_rust
    import concourse.mybir as mybir

    ctr = [0]
    for fn in nc.m.functions:
        for bb in fn.blocks:
            insts = list(bb.instructions)
            out = []
            changed = False
            for inst in insts:
                si = inst.sync_info
                waits = list(si.on_wait) if si is not None and si.on_wait else []
                sem_waits = [w for w in waits if w.sync_type == "semaphore"]
                other = [w for w in waits if w.sync_type != "semaphore"]
                budget = max_waits - len(other)
                if len(sem_waits) > budget and budget >= 1:
                    head, keep = sem_waits[:-budget], sem_waits[-budget:]
                    chunks = [
                        head[i : i + max_waits]
                        for i in range(0, len(head), max_waits)
                    ]
                    for ch in chunks:
                        nop = mybir.InstNoOp(
                            name=f"wsplit-{ctr[0]}",
                            opcode="NoOp",
                            engine=inst.engine,
                            ins=[],
                            outs=[],
                        )
                        nop.sync_info = bass_rust.SyncInfo(on_wait=ch, on_update=[])
                        ctr[0] += 1
                        out.append(nop)
                    inst.sync_info = bass_rust.SyncInfo(
                        on_wait=other + keep,
                        on_update=list(si.on_update) if si.on_update else [],
                    )
                    changed = True
                out.append(inst)
            if changed:
                bb.instructions = out


# --------------------------------------------------------------------------
# device program (identical on all 8 cores)
# --------------------------------------------------------------------------
def _build(split_waits=True):
    import concourse.bass as bass
    import concourse.tile as tile
    import concourse.mybir as mybir

    F32 = mybir.dt.float32
    F32R = mybir.dt.float32r
    BF16 = mybir.dt.bfloat16
    EXP = mybir.ActivationFunctionType.Exp
    IDENT = mybir.ActivationFunctionType.Identity
    ts = bass.ts

    nc = bass.Bass()

    xb = nc.dram_tensor("xb", [D, S], BF16, kind="ExternalInput")
    wqk = nc.dram_tensor("wqk", [D, 2 * DG], BF16, kind="ExternalInput")
    wv = nc.dram_tensor("wv", [D, DG], BF16, kind="ExternalInput")
    bqk = nc.dram_tensor("bqk", [128, 8], F32, kind="ExternalInput")
    bv = nc.dram_tensor("bv", [128, DG], F32, kind="ExternalInput")
    wo = nc.dram_tensor("wo", [DG, D], F32R, kind="ExternalInput")
    bo = nc.dram_tensor("bo", [128, D // 128], F32, kind="ExternalInput")
    outp = nc.dram_tensor("outp", [D, S], BF16, kind="ExternalOutput")

    NSQT = S // 128          # 16 sq/sk tiles of 128
    NDT = D // 128           # 8 contraction tiles
    NPAIR = HG // 2          # 4 head pairs
    VW = HD + 1              # 65: v columns + ones column per head
    SQQ = 512                # sq quarter per pv accumulation

    with tile.TileContext(nc) as tc:
        with (
            tc.tile_pool(name="bias", bufs=1) as bias_pool,
            tc.tile_pool(name="vaug", bufs=1) as v_pool,
            tc.tile_pool(name="oT", bufs=1) as oT_pool,
            tc.tile_pool(name="qkp", bufs=1) as qkp,      # rotating qT/kT slots
            tc.tile_pool(name="wqkp", bufs=1) as wqkp,
            tc.tile_pool(name="xs2", bufs=1) as xs2,      # resident bf16 x [D,S]
            tc.tile_pool(name="psqk", bufs=1, space="PSUM") as ps_qk,  # pa,pb
        ):
            bqk_t = bias_pool.tile([128, 8], F32)
            nc.gpsimd.dma_start(bqk_t[:], bqk[:, :])
            bv_t = bias_pool.tile([128, DG], F32)
            nc.gpsimd.dma_start(bv_t[:], bv[:, :])
            ones8_f = bias_pool.tile([128, 8], F32)
            nc.vector.memset(ones8_f[:], 1.0)
            ones8 = bias_pool.tile([128, 8], BF16)
            nc.vector.tensor_copy(ones8[:], ones8_f[:])
            # preload the exp table set during pass 1 (one-time ~2.7us)
            actwarm = bias_pool.tile([1, 1], F32)
            nc.scalar.activation(actwarm[:], ones8_f[0:1, 0:1], EXP)

            # wqk_t[d]: cols 0:512 = wq (SCALE folded), 512:1024 = wk
            wqk_t = [
                wqkp.tile([128, 2 * DG], BF16, name=f"wqk{d}", tag=f"wqk{d}")
                for d in range(NDT)
            ]

            v_t = [v_pool.tile([128, HG * VW], BF16, name=f"v{s}", tag=f"v{s}") for s in range(NSQT)]
            oT_t = [oT_pool.tile([128, S], F32R, name=f"oT{p}", tag=f"oT{p}") for p in range(NPAIR)]
            qT_t = [qkp.tile([128, S], BF16, name=f"qT{i}", tag=f"qT{i}") for i in range(2)]
            kT_t = [qkp.tile([128, S], BF16, name=f"kT{i}", tag=f"kT{i}") for i in range(2)]

            # ------- pass 1: single sweep over x -> v(all) + qk(pair0) -------
            # x arrives as [128,1024] bf16 tiles (2KB DMA lines), each
            # resident across the two 512-col chunks that consume it; per
            # chunk+d we issue 6 back-to-back matmuls.  PSUM: gq/gk double-
            # buffered across chunks (pa,pb | pg,ph) + 4 v banks = 8.
            with (
                tc.tile_pool(name="wvp", bufs=1) as wvp,
                tc.tile_pool(name="psv", bufs=1, space="PSUM") as ps_v,
                tc.tile_pool(name="psg", bufs=1, space="PSUM") as ps_g,
            ):
                wv_t = [wvp.tile([128, DG], BF16, name=f"wv{d}", tag=f"wv{d}") for d in range(NDT)]
                # x stays resident in SBUF for the whole kernel (the phase-B
                # fillers then need no DMA at all); weight and x loads are
                # emitted d-interleaved in demand order across both hw queues
                xr_t = [
                    [
                        xs2.tile([128, 1024], BF16, name=f"x2_{cc}_{d}", tag=f"x{cc}_{d}")
                        for d in range(NDT)
                    ]
                    for cc in range(S // 1024)
                ]
                # three concurrent DMA queues (sync/scalar/gpsimd-SWDGE
                # aggregate ~360GB/s); cc0-critical tensors first, cc1's x
                # trails so it never competes with the cc0 window
                for d in range(NDT):
                    nc.sync.dma_start(wqk_t[d][:], wqk[ts(d, 128), :])
                    nc.gpsimd.dma_start(wv_t[d][:], wv[ts(d, 128), :])
                    nc.scalar.dma_start(xr_t[0][d][:], xb[ts(d, 128), ts(0, 1024)])
                for d in range(NDT):
                    eng = nc.sync if d % 2 == 0 else nc.scalar
                    eng.dma_start(xr_t[1][d][:], xb[ts(d, 128), ts(1, 1024)])

                for cc in range(S // 1024):
                    x2_t = xr_t[cc]
                    for sub in range(2):
                        ci = 2 * cc + sub
                        if ci % 2 == 0:
                            gq = ps_qk.tile([128, 512], F32, name="gq", tag="pa")
                            gk = ps_qk.tile([128, 512], F32, name="gk", tag="pb")
                        else:
                            gq = ps_g.tile([128, 512], F32, name="gq", tag="pg")
                            gk = ps_g.tile([128, 512], F32, name="gk", tag="ph")
                        psv = [
                            ps_v.tile([128, DG], F32, name="psv", tag=t)
                            for t in ("pc", "pd", "pe", "pf")
                        ]
                        for d in range(NDT):
                            xch = x2_t[d][:, ts(sub, 512)]
                            nc.tensor.matmul(
                                gq[:], wqk_t[d][:, 0:128], xch,
                                start=(d == 0), stop=(d == NDT - 1),
                            )
                            nc.tensor.matmul(
                                gk[:], wqk_t[d][:, DG : DG + 128], xch,
                                start=(d == 0), stop=(d == NDT - 1),
                            )
                            for si in range(4):
                                nc.tensor.matmul(
                                    psv[si][:], xch[:, ts(si, 128)], wv_t[d][:],
                                    start=(d == 0), stop=(d == NDT - 1),
                                )
                        # drains: v-aug first (next chunk's psv matmuls wait
                        # on these); qk bias adds last (banks double-buffered)
                        for si in range(4):
                            s = 4 * ci + si
                            vap = v_t[s][:].rearrange("p (h e) -> p h e", e=VW)
                            nc.vector.tensor_add(
                                vap[:, :, 0:HD],
                                psv[si][:].rearrange("p (h e) -> p h e", e=HD),
                                bv_t[:].rearrange("p (h e) -> p h e", e=HD),
                            )
                            nc.gpsimd.tensor_copy(
                                vap[:, :, HD : HD + 1], ones8[:, :, None]
                            )
                        nc.vector.tensor_scalar_add(
                            qT_t[0][:, ts(ci, 512)], gq[:], bqk_t[:, 0:1]
                        )
                        nc.vector.tensor_scalar_add(
                            kT_t[0][:, ts(ci, 512)], gk[:], bqk_t[:, 4:5]
                        )

            # ---------------- phase B: attention + fillers + tail ---------
            # Two heads share one [128,1024] scores psum tile; one exp covers
            # both heads.  pv matmuls are software-pipelined one step behind.
            # The next pair's q/k projection fills the PE slack under the
            # ACT-bound exp stream; x tiles are re-read as [128,1024] bf16
            # spanning two qu's.  The out-projection tail lives inside this
            # pool scope (reusing attention PSUM tags) so no pool-exit
            # barrier precedes it.
            with (
                tc.tile_pool(name="pt", bufs=3) as ptp,
                tc.tile_pool(name="scp", bufs=2, space="PSUM") as scp,
                tc.tile_pool(name="pvp", bufs=1, space="PSUM") as pvp,
                tc.tile_pool(name="nrm", bufs=1) as nrm,
                tc.tile_pool(name="pvs", bufs=1) as pvsp,
                tc.tile_pool(name="rs", bufs=4, space="DRAM") as rsp,
                tc.tile_pool(name="w3", bufs=1) as w3,
                tc.tile_pool(name="outb", bufs=3) as outb,
            ):
                wo_t = [
                    w3.tile([128, D], F32R, name=f"wo{pp}", tag=f"wo{pp}")
                    for pp in range(NPAIR)
                ]
                for pr in range(NPAIR):
                    h0, h1 = 2 * pr, 2 * pr + 1
                    qTc, kTc = qT_t[pr % 2], kT_t[pr % 2]
                    nxt = pr + 1 if pr + 1 < NPAIR else None
                    if pr == 2:
                        # wo on sync: its phase-B program is semaphore-paced,
                        # so the transfer lands here and not during pass 1
                        # (gpsimd would issue it immediately)
                        for pp in range(NPAIR):
                            nc.sync.dma_start(wo_t[pp][:], wo[ts(pp, 128), :])
                    for qu in range(S // SQQ):
                        qs = slice(qu * SQQ, (qu + 1) * SQQ)
                        pv0 = pvp.tile([VW, SQQ], F32, name="pv0", tag="pv0")
                        pv1 = pvp.tile([VW, SQQ], F32, name="pv1", tag="pv1")
                        if nxt is not None:
                            gq = ps_qk.tile([128, 512], F32, name="gq", tag="pa")
                            gk = ps_qk.tile([128, 512], F32, name="gk", tag="pb")
                        # pv matmuls run 2 iterations behind their exp so the
                        # pv LDWEIGHTS carries no pending wait and hides
                        # behind in-flight matmuls
                        pt_q = []
                        for s in range(NSQT):
                            sc = scp.tile([128, 2 * SQQ], F32, name="sc", tag="sc")
                            nc.tensor.matmul(
                                sc[:, 0:SQQ],
                                kTc[0:HD, ts(s, 128)],
                                qTc[0:HD, qs],
                                start=True, stop=True,
                            )
                            nc.tensor.matmul(
                                sc[:, SQQ : 2 * SQQ],
                                kTc[HD:128, ts(s, 128)],
                                qTc[HD:128, qs],
                                start=True, stop=True,
                            )
                            pt = ptp.tile([128, 2 * SQQ], BF16, name="pt", tag="pt")
                            nc.scalar.activation(pt[:], sc[:], EXP)
                            if len(pt_q) == 2:
                                pp_, ps_ = pt_q.pop(0)
                                nc.tensor.matmul(
                                    pv0[:], v_t[ps_][:, h0 * VW : (h0 + 1) * VW],
                                    pp_[:, 0:SQQ],
                                    start=(ps_ == 0), stop=False,
                                )
                                nc.tensor.matmul(
                                    pv1[:], v_t[ps_][:, h1 * VW : (h1 + 1) * VW],
                                    pp_[:, SQQ : 2 * SQQ],
                                    start=(ps_ == 0), stop=False,
                                )
                            if nxt is not None:
                                # filler: qk projection for the next pair,
                                # 1 matmul/iter from the resident x tiles,
                                # finishing (with bias adds) at s==14
                                if s < 12:
                                    d = s // 2
                                    xch = xr_t[qu // 2][d][:, ts(qu % 2, 512)]
                                    if s % 2 == 0:
                                        nc.tensor.matmul(
                                            gq[:], wqk_t[d][:, ts(nxt, 128)], xch,
                                            start=(d == 0), stop=False,
                                        )
                                    else:
                                        nc.tensor.matmul(
                                            gk[:],
                                            wqk_t[d][:, DG + nxt * 128 : DG + (nxt + 1) * 128],
                                            xch,
                                            start=(d == 0), stop=False,
                                        )
                                elif s in (12, 13):
                                    d = s - 6
                                    xch = xr_t[qu // 2][d][:, ts(qu % 2, 512)]
                                    nc.tensor.matmul(
                                        gq[:], wqk_t[d][:, ts(nxt, 128)], xch,
                                        start=False, stop=(d == NDT - 1),
                                    )
                                    nc.tensor.matmul(
                                        gk[:],
                                        wqk_t[d][:, DG + nxt * 128 : DG + (nxt + 1) * 128],
                                        xch,
                                        start=False, stop=(d == NDT - 1),
                                    )
                                    if s == 13:
                                        nc.vector.tensor_scalar_add(
                                            qT_t[nxt % 2][:, ts(qu, 512)], gq[:],
                                            bqk_t[:, nxt : nxt + 1],
                                        )
                                        nc.vector.tensor_scalar_add(
                                            kT_t[nxt % 2][:, ts(qu, 512)], gk[:],
                                            bqk_t[:, 4 + nxt : 5 + nxt],
                                        )
                            pt_q.append((pt, s))
                        for pp_, ps_ in pt_q:
                            nc.tensor.matmul(
                                pv0[:], v_t[ps_][:, h0 * VW : (h0 + 1) * VW],
                                pp_[:, 0:SQQ],
                                start=False, stop=(ps_ == NSQT - 1),
                            )
                            nc.tensor.matmul(
                                pv1[:], v_t[ps_][:, h1 * VW : (h1 + 1) * VW],
                                pp_[:, SQQ : 2 * SQQ],
                                start=False, stop=(ps_ == NSQT - 1),
                            )
                        # free pv banks via psum->sbuf copy, then normalize.
                        # The denominator row [1,512] is reshaped to [128,4]
                        # through DRAM so the DVE reciprocal costs ~0.1us
                        # instead of 3.3us; the multiply runs on GPSIMD.
                        for hh, pvx, row in ((0, pv0, 0), (1, pv1, HD)):
                            pvs = pvsp.tile([VW, SQQ], F32, name=f"pvs{hh}", tag=f"pvs{hh}")
                            nc.vector.tensor_copy(pvs[:], pvx[:])
                            rsd = rsp.tile([1, SQQ], F32, name=f"rsd{hh}", tag=f"rsd{hh}")
                            nc.sync.dma_start(rsd[:], pvs[HD : HD + 1, :])
                            dent = nrm.tile([128, 4], F32, name=f"dent{hh}", tag=f"dent{hh}")
                            nc.sync.dma_start(dent[:], rsd[:])
                            rcd = nrm.tile([128, 4], F32, name=f"rcd{hh}", tag=f"rcd{hh}")
                            nc.vector.reciprocal(rcd[:], dent[:])
                            rs2 = rsp.tile([1, SQQ], F32, name=f"rs2{hh}", tag=f"rs2{hh}")
                            nc.sync.dma_start(rs2[:], rcd[:])
                            bcs = nrm.tile([HD, SQQ], F32, name=f"bcs{hh}", tag=f"bcs{hh}")
                            nc.sync.dma_start(bcs[:], rs2[:].broadcast_to([HD, SQQ]))
                            nc.gpsimd.tensor_mul(
                                oT_t[pr][row : row + HD, qs], pvs[0:HD, :], bcs[:]
                            )

                # ---------------- tail: out projection (transposed) -------
                # outT[j,q] = sum_pp wo_pp[:,j]^T @ oT_pp[:,q]: wo stationary
                # (reused over 4 moving q-chunks), bias as per-partition
                # scalar on the now-idle ACT engine, host transposes [D,S].
                # PSUM comes from freed attention tags; q-chunk 3 (which
                # waits on the last qu's normalization) goes last.
                bo_t = outb.tile([128, NDT], F32, name="bo_t", tag="bo_t")
                nc.gpsimd.dma_start(bo_t[:], bo[:, :])
                po_slots = [
                    (scp, "sc"), (ps_qk, "pa"), (ps_qk, "pb"),
                    (pvp, "pv0"), (pvp, "pv1"), (scp, "sc"),
                ]
                nunit = 0

                def _unit2(j, tcpair):
                    # two q-chunks -> one [128,1024] bf16 tile -> one DMA
                    # (2KB lines; bf16 halves the 8MB output stream)
                    nonlocal nunit
                    ob = outb.tile([128, 1024], BF16, name=f"ob{j}_{tcpair}", tag="ob")
                    for half in range(2):
                        tc_ = 2 * tcpair + half
                        pool, tag = po_slots[nunit % len(po_slots)]
                        po = pool.tile([128, 512], F32, name=f"po{j}_{tc_}", tag=tag)
                        for pp in range(NPAIR):
                            nc.tensor.matmul(
                                po[:],
                                wo_t[pp][:, ts(j, 128)],
                                oT_t[pp][:, ts(tc_, 512)],
                                start=(pp == 0), stop=(pp == NPAIR - 1),
                            )
                        if nunit % 2 == 0:
                            nc.scalar.activation(
                                ob[:, ts(half, 512)], po[:], IDENT,
                                bias=bo_t[:, j : j + 1],
                            )
                        else:
                            nc.vector.tensor_scalar_add(
                                ob[:, ts(half, 512)], po[:], bo_t[:, j : j + 1]
                            )
                        nunit += 1
                    nc.sync.dma_start(outp[ts(j, 128), ts(tcpair, 1024)], ob[:])

                for j in range(NDT):
                    _unit2(j, 0)
                for j in range(NDT):
                    _unit2(j, 1)

    if split_waits:
        _split_excess_waits(nc, max_waits=1)
    return nc


def _get_nc():
    if "nc" not in _CACHE:
        _CACHE["nc"] = _build()
    return _CACHE["nc"]


# --------------------------------------------------------------------------
# host entry point
# --------------------------------------------------------------------------
def _shard_inputs(x, w_qkv, b_qkv, w_out, b_out):
    import ml_dtypes

    f = np.float32
    bf = np.dtype(ml_dtypes.bfloat16)
    x = np.asarray(x, f)
    w_qkv = np.asarray(w_qkv, f)
    b_qkv = np.asarray(b_qkv, f)
    w_out = np.asarray(w_out, f)
    b_out = np.asarray(b_out, f)
    in_maps = []
    for c in range(NCORES):
        b, g = divmod(c, 2)
        cols = slice(DG * g, DG * (g + 1))
        wq_c = w_qkv[:, 0 * D :][:, cols][:, :DG] * np.float32(SCALE)
        wk_c = w_qkv[:, D : 2 * D][:, cols]
        wqk_c = np.ascontiguousarray(
            np.concatenate([wq_c, wk_c], axis=1).astype(bf)
        )
        wv_c = np.ascontiguousarray(w_qkv[:, 2 * D :][:, cols].astype(bf))
        bq_c = (b_qkv[0 * D : 1 * D][cols] * np.float32(SCALE)).reshape(4, 128).T
        bk_c = b_qkv[D : 2 * D][cols].reshape(4, 128).T
        bqk_c = np.ascontiguousarray(np.concatenate([bq_c, bk_c], axis=1), f)
        bv_c = np.ascontiguousarray(np.tile(b_qkv[2 * D :][cols], (128, 1)), f)
        wo_c = np.ascontiguousarray(w_out[DG * g : DG * (g + 1), :])
        bo_c = (
            np.ascontiguousarray(b_out.reshape(D // 128, 128).T, f)
            if g == 0
            else np.zeros((128, D // 128), f)
        )
        in_maps.append(
            {
                "xb": np.ascontiguousarray(x[b].T.astype(bf)),
                "wqk": wqk_c,
                "wv": wv_c,
                "bqk": bqk_c,
                "bv": bv_c,
                "wo": wo_c,
                "bo": bo_c,
            }
        )
    return in_maps


def _patch_ldw_opt():
    """Flip walrus --enable-ldw-opt to true (dedupe repeated LDWEIGHTS for
    consecutive same-stationary matmuls). Off by default: the bf16 matmuls
    now lower to standalone InstLdweights, which walrus rejects under
    ldw-opt. Controlled by KERNEL_LDW_OPT env."""
    import os
    if os.environ.get("KERNEL_LDW_OPT", "0") != "1":
        return
    if _CACHE.get("ldw_patched"):
        return
    import concourse.bass_utils as bu

    orig = bu.run_command

    def run_command_ldw(argv, **kwargs):
        argv = [a.replace("--enable-ldw-opt=false", "--enable-ldw-opt=true")
                if isinstance(a, str) else a for a in argv]
        return orig(argv, **kwargs)

    bu.run_command = run_command_ldw
    _CACHE["ldw_patched"] = True


def kernel(x, w_qkv, b_qkv, w_out, b_out, _trace=False, _trace_kwargs=None):
    from concourse.bass_utils import run_bass_kernel_spmd

    _patch_ldw_opt()
    nc = _get_nc()
    in_maps = _shard_inputs(x, w_qkv, b_qkv, w_out, b_out)
    kw = {}
    if _trace:
        kw["trace"] = True
        kw.update(_trace_kwargs or {})
    res = run_bass_kernel_spmd(nc, in_maps, core_ids=list(range(NCORES)), **kw)
    _CACHE["last_result"] = res
    # [D, S] bf16 per core
    parts = [np.asarray(r["outp"], dtype=np.float32) for r in res.results]
    out = np.stack([(parts[2 * b] + parts[2 * b + 1]).T for b in range(B)])
    return np.ascontiguousarray(out, np.float32)


# revision 45
# speedup vs baseline: 1.0267x; 1.0029x over previous
"""Multi-head attention (B=4, S=2048, D=1024, H=16) on 8 Trainium2 NeuronCores.

Sharding: core c -> batch c//2, head-group c%2 (8 heads = 512 dims each).
Each core computes qkv projection, softmax attention and its partial
out-projection (Megatron row-split of w_out); the host sums core pairs.

All projection/attention operands are bf16 with fp32 PSUM accumulation
(x, qkv weights, qT/kT, v, exp(scores)); only the out-projection stays
float32r.  bf16 halves DMA bytes (the DMA engines are packet-rate bound:
~288ns per <=2KB line, so all big transfers use >=2KB lines) and makes
every hot-loop LDWEIGHTS a standalone fast-load that hides behind
matmuls.  x lives resident in SBUF (4MB bf16) so phase-B fillers do no
DMA.  Softmax needs no max-subtraction (scores ~ N(0,1)); denominators
come free from an augmented ones-column in V; the reciprocal runs on a
[128,4] DRAM-reshaped layout (DVE reciprocal cost scales with free
size), and the normalization multiply runs on the idle GPSIMD.

Schedule (profiled ~372us, exp-ACT-bound):
  - pass 1 (~58us): one sweep over resident x computes v (all heads) +
    q/k (pair 0), six back-to-back matmuls per x tile; loads spread over
    all three DMA-capable queues (sync/scalar/gpsimd run concurrently,
    ~360GB/s aggregate vs ~128GB/s each); psum drains double-buffered so
    chunk boundaries never stall the PE.
  - phase B (~274us, ACT ~99% busy): per head-pair, scores (row-tiled
    2-head pack) -> one exp per [128,1024] psum tile -> pv accumulation
    lagging 2 iterations (pv LDWEIGHTS carries no pending wait); filler
    matmuls project the next pair's q/k from resident x in the PE slack,
    finishing 2 iterations before the qu boundary.
  - tail (~29us): transposed out-projection (wo stationary, output [D,S]
    bf16, host transposes back), PSUM from freed attention tags (no pool
    barrier), bias adds alternating ACT/DVE, paired [128,1024] output
    DMAs.
"""

import numpy as np

B, S, D, H = 4, 2048, 1024, 16
HD = D // H          # 64
HG = H // 2          # 8 heads per core
DG = HG * HD         # 512 local head-cat dims
SCALE = HD ** -0.5   # folded into wq host-side
NCORES = 8

_CACHE = {}


# --------------------------------------------------------------------------
# wait splitting: this toolchain's walrus rejects >1 sync wait per instruction
# on some paths; move excess semaphore waits onto same-engine NoOps.
# --------------------------------------------------------------------------
def _split_excess_waits(nc, max_waits=1):
    import bass_rust
    import concourse.mybir as mybir

    ctr = [0]
    for fn in nc.m.functions:
        for bb in fn.blocks:
            insts = list(bb.instructions)
            out = []
            changed = False
            for inst in insts:
                si = inst.sync_info
                waits = list(si.on_wait) if si is not None and si.on_wait else []
                sem_waits = [w for w in waits if w.sync_type == "semaphore"]
                other = [w for w in waits if w.sync_type != "semaphore"]
                budget = max_waits - len(other)
                if len(sem_waits) > budget and budget >= 1:
                    head, keep = sem_waits[:-budget], sem_waits[-budget:]
                    chunks = [
                        head[i : i + max_waits]
                        for i in range(0, len(head), max_waits)
                    ]
                    for ch in chunks:
                        nop = mybir.InstNoOp(
                            name=f"wsplit-{ctr[0]}",
                            opcode="NoOp",
                            engine=inst.engine,
                            ins=[],
                            outs=[],
                        )
                        nop.sync_info = bass_rust.SyncInfo(on_wait=ch, on_update=[])
                        ctr[0] += 1
                        out.append(nop)
                    inst.sync_info = bass_rust.SyncInfo(
                        on_wait=other + keep,
                        on_update=list(si.on_update) if si.on_update else [],
                    )
                    changed = True
                out.append(inst)
            if changed:
                bb.instructions = out


# --------------------------------------------------------------------------
# device program (identical on all 8 cores)
# --------------------------------------------------------------------------
def _build(split_waits=True):
    import concourse.bass as bass
    import concourse.tile as tile
    import concourse.mybir as mybir

    F32 = mybir.dt.float32
    F32R = mybir.dt.float32r
    BF16 = mybir.dt.bfloat16
    EXP = mybir.ActivationFunctionType.Exp
    IDENT = mybir.ActivationFunctionType.Identity
    ts = bass.ts

    nc = bass.Bass()

    xb = nc.dram_tensor("xb", [D, S], BF16, kind="ExternalInput")
    wqk = nc.dram_tensor("wqk", [D, 2 * DG], BF16, kind="ExternalInput")
    wv = nc.dram_tensor("wv", [D, DG], BF16, kind="ExternalInput")
    bqk = nc.dram_tensor("bqk", [128, 8], F32, kind="ExternalInput")
    bv = nc.dram_tensor("bv", [128, DG], F32, kind="ExternalInput")
    wo = nc.dram_tensor("wo", [DG, D], F32R, kind="ExternalInput")
    bo = nc.dram_tensor("bo", [128, D // 128], F32, kind="ExternalInput")
    outp = nc.dram_tensor("outp", [D, S], BF16, kind="ExternalOutput")

    NSQT = S // 128          # 16 sq/sk tiles of 128
    NDT = D // 128           # 8 contraction tiles
    NPAIR = HG // 2          # 4 head pairs
    VW = HD + 1              # 65: v columns + ones column per head
    SQQ = 512                # sq quarter per pv accumulation

    with tile.TileContext(nc) as tc:
        with (
            tc.tile_pool(name="bias", bufs=1) as bias_pool,
            tc.tile_pool(name="vaug", bufs=1) as v_pool,
            tc.tile_pool(name="oT", bufs=1) as oT_pool,
            tc.tile_pool(name="qkp", bufs=1) as qkp,      # rotating qT/kT slots
            tc.tile_pool(name="wqkp", bufs=1) as wqkp,
            tc.tile_pool(name="xs2", bufs=1) as xs2,      # resident bf16 x [D,S]
            tc.tile_pool(name="psqk", bufs=1, space="PSUM") as ps_qk,  # pa,pb
        ):
            bqk_t = bias_pool.tile([128, 8], F32)
            bv_t = bias_pool.tile([128, DG], F32)
            ones8_f = bias_pool.tile([128, 8], F32)
            nc.vector.memset(ones8_f[:], 1.0)
            ones8 = bias_pool.tile([128, 8], BF16)
            nc.vector.tensor_copy(ones8[:], ones8_f[:])
            actwarm = bias_pool.tile([1, 1], F32)

            # wqk_t[d]: cols 0:512 = wq (SCALE folded), 512:1024 = wk
            wqk_t = [
                wqkp.tile([128, 2 * DG], BF16, name=f"wqk{d}", tag=f"wqk{d}")
                for d in range(NDT)
            ]

            v_t = [v_pool.tile([128, HG * VW], BF16, name=f"v{s}", tag=f"v{s}") for s in range(NSQT)]
            oT_t = [oT_pool.tile([128, S], F32R, name=f"oT{p}", tag=f"oT{p}") for p in range(NPAIR)]
            qT_t = [qkp.tile([128, S], BF16, name=f"qT{i}", tag=f"qT{i}") for i in range(2)]
            kT_t = [qkp.tile([128, S], BF16, name=f"kT{i}", tag=f"kT{i}") for i in range(2)]

            # ------- pass 1: single sweep over x -> v(all) + qk(pair0) -------
            # x arrives as [128,1024] bf16 tiles (2KB DMA lines), each
            # resident across the two 512-col chunks that consume it; per
            # chunk+d we issue 6 back-to-back matmuls.  PSUM: gq/gk double-
            # buffered across chunks (pa,pb | pg,ph) + 4 v banks = 8.
            with (
                tc.tile_pool(name="wvp", bufs=1) as wvp,
                tc.tile_pool(name="psv", bufs=1, space="PSUM") as ps_v,
                tc.tile_pool(name="psg", bufs=1, space="PSUM") as ps_g,
            ):
                wv_t = [wvp.tile([128, DG], BF16, name=f"wv{d}", tag=f"wv{d}") for d in range(NDT)]
                # x stays resident in SBUF for the whole kernel (the phase-B
                # fillers then need no DMA at all); weight and x loads are
                # emitted d-interleaved in demand order across both hw queues
                xr_t = [
                    [
                        xs2.tile([128, 1024], BF16, name=f"x2_{cc}_{d}", tag=f"x{cc}_{d}")
                        for d in range(NDT)
                    ]
                    for cc in range(S // 1024)
                ]
                # three concurrent DMA queues (sync/scalar/gpsimd-SWDGE
                # aggregate ~360GB/s); cc0-critical tensors first, cc1's x
                # trails so it never competes with the cc0 window
                for d in range(NDT):
                    nc.sync.dma_start(wqk_t[d][:], wqk[ts(d, 128), :])
                    nc.gpsimd.dma_start(wv_t[d][:], wv[ts(d, 128), :])
                    nc.scalar.dma_start(xr_t[0][d][:], xb[ts(d, 128), ts(0, 1024)])
                for d in range(NDT):
                    eng = nc.sync if d % 2 == 0 else nc.scalar
                    eng.dma_start(xr_t[1][d][:], xb[ts(d, 128), ts(1, 1024)])
                # non-critical loads behind the pass-1 stream: biases (first
                # needed at the ci=0 drain) and the one-time exp table
                # preload (~2.7us, must land before phase B)
                nc.gpsimd.dma_start(bqk_t[:], bqk[:, :])
                nc.gpsimd.dma_start(bv_t[:], bv[:, :])
                nc.scalar.activation(actwarm[:], ones8_f[0:1, 0:1], EXP)

                for cc in range(S // 1024):
                    x2_t = xr_t[cc]
                    for sub in range(2):
                        ci = 2 * cc + sub
                        if ci % 2 == 0:
                            gq = ps_qk.tile([128, 512], F32, name="gq", tag="pa")
                            gk = ps_qk.tile([128, 512], F32, name="gk", tag="pb")
                        else:
                            gq = ps_g.tile([128, 512], F32, name="gq", tag="pg")
                            gk = ps_g.tile([128, 512], F32, name="gk", tag="ph")
                        psv = [
                            ps_v.tile([128, DG], F32, name="psv", tag=t)
                            for t in ("pc", "pd", "pe", "pf")
                        ]
                        for d in range(NDT):
                            xch = x2_t[d][:, ts(sub, 512)]
                            nc.tensor.matmul(
                                gq[:], wqk_t[d][:, 0:128], xch,
                                start=(d == 0), stop=(d == NDT - 1),
                            )
                            nc.tensor.matmul(
                                gk[:], wqk_t[d][:, DG : DG + 128], xch,
                                start=(d == 0), stop=(d == NDT - 1),
                            )
                            for si in range(4):
                                nc.tensor.matmul(
                                    psv[si][:], xch[:, ts(si, 128)], wv_t[d][:],
                                    start=(d == 0), stop=(d == NDT - 1),
                                )
                        # drains: v-aug first (next chunk's psv matmuls wait
                        # on these); qk bias adds last (banks double-buffered)
                        for si in range(4):
                            s = 4 * ci + si
                            vap = v_t[s][:].rearrange("p (h e) -> p h e", e=VW)
                            nc.vector.tensor_add(
                                vap[:, :, 0:HD],
                                psv[si][:].rearrange("p (h e) -> p h e", e=HD),
                                bv_t[:].rearrange("p (h e) -> p h e", e=HD),
                            )
                            nc.gpsimd.tensor_copy(
                                vap[:, :, HD : HD + 1], ones8[:, :, None]
                            )
                        nc.vector.tensor_scalar_add(
                            qT_t[0][:, ts(ci, 512)], gq[:], bqk_t[:, 0:1]
                        )
                        nc.vector.tensor_scalar_add(
                            kT_t[0][:, ts(ci, 512)], gk[:], bqk_t[:, 4:5]
                        )

            # ---------------- phase B: attention + fillers + tail ---------
            # Two heads share one [128,1024] scores psum tile; one exp covers
            # both heads.  pv matmuls are software-pipelined one step behind.
            # The next pair's q/k projection fills the PE slack under the
            # ACT-bound exp stream; x tiles are re-read as [128,1024] bf16
            # spanning two qu's.  The out-projection tail lives inside this
            # pool scope (reusing attention PSUM tags) so no pool-exit
            # barrier precedes it.
            with (
                tc.tile_pool(name="pt", bufs=3) as ptp,
                tc.tile_pool(name="scp", bufs=2, space="PSUM") as scp,
                tc.tile_pool(name="pvp", bufs=1, space="PSUM") as pvp,
                tc.tile_pool(name="nrm", bufs=1) as nrm,
                tc.tile_pool(name="pvs", bufs=1) as pvsp,
                tc.tile_pool(name="rs", bufs=4, space="DRAM") as rsp,
                tc.tile_pool(name="w3", bufs=1) as w3,
                tc.tile_pool(name="outb", bufs=3) as outb,
            ):
                wo_t = [
                    w3.tile([128, D], F32R, name=f"wo{pp}", tag=f"wo{pp}")
                    for pp in range(NPAIR)
                ]
                for pr in range(NPAIR):
                    h0, h1 = 2 * pr, 2 * pr + 1
                    qTc, kTc = qT_t[pr % 2], kT_t[pr % 2]
                    nxt = pr + 1 if pr + 1 < NPAIR else None
                    if pr == 2:
                        # wo on sync: its phase-B program is semaphore-paced,
                        # so the transfer lands here and not during pass 1
                        # (gpsimd would issue it immediately)
                        for pp in range(NPAIR):
                            nc.sync.dma_start(wo_t[pp][:], wo[ts(pp, 128), :])
                    for qu in range(S // SQQ):
                        qs = slice(qu * SQQ, (qu + 1) * SQQ)
                        pv0 = pvp.tile([VW, SQQ], F32, name="pv0", tag="pv0")
                        pv1 = pvp.tile([VW, SQQ], F32, name="pv1", tag="pv1")
                        if nxt is not None:
                            gq = ps_qk.tile([128, 512], F32, name="gq", tag="pa")
                            gk = ps_qk.tile([128, 512], F32, name="gk", tag="pb")
                        # pv matmuls run 2 iterations behind their exp so the
                        # pv LDWEIGHTS carries no pending wait and hides
                        # behind in-flight matmuls
                        pt_q = []
                        for s in range(NSQT):
                            sc = scp.tile([128, 2 * SQQ], F32, name="sc", tag="sc")
                            nc.tensor.matmul(
                                sc[:, 0:SQQ],
                                kTc[0:HD, ts(s, 128)],
                                qTc[0:HD, qs],
                                start=True, stop=True,
                            )
                            nc.tensor.matmul(
                                sc[:, SQQ : 2 * SQQ],
                                kTc[HD:128, ts(s, 128)],
                                qTc[HD:128, qs],
                                start=True, stop=True,
                            )
                            pt = ptp.tile([128, 2 * SQQ], BF16, name="pt", tag="pt")
                            nc.scalar.activation(pt[:], sc[:], EXP)
                            if len(pt_q) == 2:
                                pp_, ps_ = pt_q.pop(0)
                                nc.tensor.matmul(
                                    pv0[:], v_t[ps_][:, h0 * VW : (h0 + 1) * VW],
                                    pp_[:, 0:SQQ],
                                    start=(ps_ == 0), stop=False,
                                )
                                nc.tensor.matmul(
                                    pv1[:], v_t[ps_][:, h1 * VW : (h1 + 1) * VW],
                                    pp_[:, SQQ : 2 * SQQ],
                                    start=(ps_ == 0), stop=False,
                                )
                            if nxt is not None:
                                # filler: qk projection for the next pair,
                                # 1 matmul/iter from the resident x tiles,
                                # finishing (with bias adds) at s==14
                                if s < 12:
                                    d = s // 2
                                    xch = xr_t[qu // 2][d][:, ts(qu % 2, 512)]
                                    if s % 2 == 0:
                                        nc.tensor.matmul(
                                            gq[:], wqk_t[d][:, ts(nxt, 128)], xch,
                                            start=(d == 0), stop=False,
                                        )
                                    else:
                                        nc.tensor.matmul(
                                            gk[:],
                                            wqk_t[d][:, DG + nxt * 128 : DG + (nxt + 1) * 128],
                                            xch,
                                            start=(d == 0), stop=False,
                                        )
                                elif s in (12, 13):
                                    d = s - 6
                                    xch = xr_t[qu // 2][d][:, ts(qu % 2, 512)]
                                    nc.tensor.matmul(
                                        gq[:], wqk_t[d][:, ts(nxt, 128)], xch,
                                        start=False, stop=(d == NDT - 1),
                                    )
                                    nc.tensor.matmul(
                                        gk[:],
                                        wqk_t[d][:, DG + nxt * 128 : DG + (nxt + 1) * 128],
                                        xch,
                                        start=False, stop=(d == NDT - 1),
                                    )
                                    if s == 13:
                                        nc.vector.tensor_scalar_add(
                                            qT_t[nxt % 2][:, ts(qu, 512)], gq[:],
                                            bqk_t[:, nxt : nxt + 1],
                                        )
                                        nc.vector.tensor_scalar_add(
                                            kT_t[nxt % 2][:, ts(qu, 512)], gk[:],
                                            bqk_t[:, 4 + nxt : 5 + nxt],
                                        )
                            pt_q.append((pt, s))
                        for pp_, ps_ in pt_q:
                            nc.tensor.matmul(
                                pv0[:], v_t[ps_][:, h0 * VW : (h0 + 1) * VW],
                                pp_[:, 0:SQQ],
                                start=False, stop=(ps_ == NSQT - 1),
                            )
                            nc.tensor.matmul(
                                pv1[:], v_t[ps_][:, h1 * VW : (h1 + 1) * VW],
                                pp_[:, SQQ : 2 * SQQ],
                                start=False, stop=(ps_ == NSQT - 1),
                            )
                        # free pv banks via psum->sbuf copy, then normalize.
                        # The denominator row [1,512] is reshaped to [128,4]
                        # through DRAM so the DVE reciprocal costs ~0.1us
                        # instead of 3.3us; the multiply runs on GPSIMD.
                        for hh, pvx, row in ((0, pv0, 0), (1, pv1, HD)):
                            pvs = pvsp.tile([VW, SQQ], F32, name=f"pvs{hh}", tag=f"pvs{hh}")
                            nc.vector.tensor_copy(pvs[:], pvx[:])
                            rsd = rsp.tile([1, SQQ], F32, name=f"rsd{hh}", tag=f"rsd{hh}")
                            nc.sync.dma_start(rsd[:], pvs[HD : HD + 1, :])
                            dent = nrm.tile([128, 4], F32, name=f"dent{hh}", tag=f"dent{hh}")
                            nc.sync.dma_start(dent[:], rsd[:])
                            rcd = nrm.tile([128, 4], F32, name=f"rcd{hh}", tag=f"rcd{hh}")
                            nc.vector.reciprocal(rcd[:], dent[:])
                            rs2 = rsp.tile([1, SQQ], F32, name=f"rs2{hh}", tag=f"rs2{hh}")
                            nc.sync.dma_start(rs2[:], rcd[:])
                            bcs = nrm.tile([HD, SQQ], F32, name=f"bcs{hh}", tag=f"bcs{hh}")
                            nc.sync.dma_start(bcs[:], rs2[:].broadcast_to([HD, SQQ]))
                            nc.gpsimd.tensor_mul(
                                oT_t[pr][row : row + HD, qs], pvs[0:HD, :], bcs[:]
                            )

                # ---------------- tail: out projection (transposed) -------
                # outT[j,q] = sum_pp wo_pp[:,j]^T @ oT_pp[:,q]: wo stationary
                # (reused over 4 moving q-chunks), bias as per-partition
                # scalar on the now-idle ACT engine, host transposes [D,S].
                # PSUM comes from freed attention tags; q-chunk 3 (which
                # waits on the last qu's normalization) goes last.
                bo_t = outb.tile([128, NDT], F32, name="bo_t", tag="bo_t")
                nc.gpsimd.dma_start(bo_t[:], bo[:, :])
                po_slots = [
                    (scp, "sc"), (ps_qk, "pa"), (ps_qk, "pb"),
                    (pvp, "pv0"), (pvp, "pv1"), (scp, "sc"),
                ]
                nunit = 0

                def _unit2(j, tcpair):
                    # two q-chunks -> one [128,1024] bf16 tile -> one DMA
                    # (2KB lines; bf16 halves the 8MB output stream)
                    nonlocal nunit
                    ob = outb.tile([128, 1024], BF16, name=f"ob{j}_{tcpair}", tag="ob")
                    for half in range(2):
                        tc_ = 2 * tcpair + half
                        pool, tag = po_slots[nunit % len(po_slots)]
                        po = pool.tile([128, 512], F32, name=f"po{j}_{tc_}", tag=tag)
                        for pp in range(NPAIR):
                            nc.tensor.matmul(
                                po[:],
                                wo_t[pp][:, ts(j, 128)],
                                oT_t[pp][:, ts(tc_, 512)],
                                start=(pp == 0), stop=(pp == NPAIR - 1),
                            )
                        if nunit % 2 == 0:
                            nc.scalar.activation(
                                ob[:, ts(half, 512)], po[:], IDENT,
                                bias=bo_t[:, j : j + 1],
                            )
                        else:
                            nc.vector.tensor_scalar_add(
                                ob[:, ts(half, 512)], po[:], bo_t[:, j : j + 1]
                            )
                        nunit += 1
                    nc.sync.dma_start(outp[ts(j, 128), ts(tcpair, 1024)], ob[:])

                for j in range(NDT):
                    _unit2(j, 0)
                for j in range(NDT):
                    _unit2(j, 1)

    if split_waits:
        _split_excess_waits(nc, max_waits=1)
    return nc


def _get_nc():
    if "nc" not in _CACHE:
        _CACHE["nc"] = _build()
    return _CACHE["nc"]


# --------------------------------------------------------------------------
# host entry point
# --------------------------------------------------------------------------
def _shard_inputs(x, w_qkv, b_qkv, w_out, b_out):
    import ml_dtypes

    f = np.float32
    bf = np.dtype(ml_dtypes.bfloat16)
    x = np.asarray(x, f)
    w_qkv = np.asarray(w_qkv, f)
    b_qkv = np.asarray(b_qkv, f)
    w_out = np.asarray(w_out, f)
    b_out = np.asarray(b_out, f)
    in_maps = []
    for c in range(NCORES):
        b, g = divmod(c, 2)
        cols = slice(DG * g, DG * (g + 1))
        wq_c = w_qkv[:, 0 * D :][:, cols][:, :DG] * np.float32(SCALE)
        wk_c = w_qkv[:, D : 2 * D][:, cols]
        wqk_c = np.ascontiguousarray(
            np.concatenate([wq_c, wk_c], axis=1).astype(bf)
        )
        wv_c = np.ascontiguousarray(w_qkv[:, 2 * D :][:, cols].astype(bf))
        bq_c = (b_qkv[0 * D : 1 * D][cols] * np.float32(SCALE)).reshape(4, 128).T
        bk_c = b_qkv[D : 2 * D][cols].reshape(4, 128).T
        bqk_c = np.ascontiguousarray(np.concatenate([bq_c, bk_c], axis=1), f)
        bv_c = np.ascontiguousarray(np.tile(b_qkv[2 * D :][cols], (128, 1)), f)
        wo_c = np.ascontiguousarray(w_out[DG * g : DG * (g + 1), :])
        bo_c = (
            np.ascontiguousarray(b_out.reshape(D // 128, 128).T, f)
            if g == 0
            else np.zeros((128, D // 128), f)
        )
        in_maps.append(
            {
                "xb": np.ascontiguousarray(x[b].T.astype(bf)),
                "wqk": wqk_c,
                "wv": wv_c,
                "bqk": bqk_c,
                "bv": bv_c,
                "wo": wo_c,
                "bo": bo_c,
            }
        )
    return in_maps


def _patch_ldw_opt():
    """Flip walrus --enable-ldw-opt to true (dedupe repeated LDWEIGHTS for
    consecutive same-stationary matmuls). Off by default: the bf16 matmuls
    now lower to standalone InstLdweights, which walrus rejects under
    ldw-opt. Controlled by KERNEL_LDW_OPT env."""
    import os
    if os.environ.get("KERNEL_LDW_OPT", "0") != "1":
        return
    if _CACHE.get("ldw_patched"):
        return
    import concourse.bass_utils as bu

    orig = bu.run_command

    def run_command_ldw(argv, **kwargs):
        argv = [a.replace("--enable-ldw-opt=false", "--enable-ldw-opt=true")
                if isinstance(a, str) else a for a in argv]
        return orig(argv, **kwargs)

    bu.run_command = run_command_ldw
    _CACHE["ldw_patched"] = True


def kernel(x, w_qkv, b_qkv, w_out, b_out, _trace=False, _trace_kwargs=None):
    from concourse.bass_utils import run_bass_kernel_spmd

    _patch_ldw_opt()
    nc = _get_nc()
    in_maps = _shard_inputs(x, w_qkv, b_qkv, w_out, b_out)
    kw = {}
    if _trace:
        kw["trace"] = True
        kw.update(_trace_kwargs or {})
    res = run_bass_kernel_spmd(nc, in_maps, core_ids=list(range(NCORES)), **kw)
    _CACHE["last_result"] = res
    # [D, S] bf16 per core
    parts = [np.asarray(r["outp"], dtype=np.float32) for r in res.results]
    out = np.stack([(parts[2 * b] + parts[2 * b + 1]).T for b in range(B)])
    return np.ascontiguousarray(out, np.float32)
